# revision 23
# baseline (speedup 1.0000x reference)
"""Trainium2 Bass kernel for ApplyAttentionPolicyMap.

out = concat(logits.reshape(B, 4096), pp_logits.reshape(B, 192)) @ fc1
where fc1 is a fixed 4288x1858 one-hot column-selection map (each output
column copies exactly one input feature; source indices strictly increasing).

Strategy (pure data parallel, batch sharded 8 ways):
  Per 128-row batch tile:
    - DMA the 4288 features to SBUF (optionally casting f32->bf16 in the DMA)
    - For each 128-wide feature tile: TensorE transpose into PSUM, evacuate
      to SBUF (features-on-partitions), then one small matmul against the
      one-hot selection slice -> output PSUM at the mapped column range.
      Since the selection is monotone, each feature tile feeds one contiguous,
      disjoint output column range - no PSUM accumulation across tiles.
    - Evacuate output PSUM to SBUF, DMA to DRAM.
"""

import numpy as np
import ml_dtypes

import concourse.bass as bass
import concourse.tile as tile
from concourse import bacc, mybir
from concourse.bass_utils import run_bass_kernel_spmd

N_CORES = 8
B = 8192
B_CORE = B // N_CORES          # 1024 rows per core
P = 128                        # partition tile
NBT = B_CORE // P              # 8 batch tiles per core
F = 4288                       # 4096 + 192 features
KT = (F + P - 1) // P          # 34 feature tiles (last one 64 wide)
NPOL = 1858                    # output columns
GROUP = 8                      # feature tiles per PSUM transpose bank (bf16: 8*128*2B = 2KB = 1 bank)

COMPUTE_DT = mybir.dt.bfloat16   # knob: bfloat16 or float32
OUT_DT = mybir.dt.bfloat16       # on-device output store; host upcasts to f32
_NP_DT = {mybir.dt.bfloat16: ml_dtypes.bfloat16,
          mybir.dt.float32: np.float32}


def _make_src():
    """Source feature index for each of the 1858 policy columns (from the
    reference's fixed policy map; one-hot per column, strictly increasing)."""
    move = np.arange(1, 8)
    diag = np.array([move + move * 8, move - move * 8,
                     move * -1 - move * 8, move * -1 + move * 8])
    knight = np.array([[2 + 1 * 8], [2 - 1 * 8], [1 - 2 * 8], [-1 - 2 * 8],
                       [-2 - 1 * 8], [-2 + 1 * 8], [-1 + 2 * 8], [1 + 2 * 8]])
    orthog = np.array([move, move * -8, move * -1, move * 8])
    promos = np.array([2 * 8, 3 * 8, 4 * 8])
    pawn_promotion = np.array([-1 + promos, 0 + promos, 1 + promos])
    traversable = []
    for i in range(8):
        for j in range(8):
            sq = 8 * i + j
            traversable.append(sq + np.sort(np.int32(np.concatenate((
                orthog[0][:7 - j], orthog[2][:j], orthog[1][:i], orthog[3][:7 - i],
                diag[0][:min(7 - i, 7 - j)], diag[3][:min(7 - i, j)],
                diag[1][:min(i, 7 - j)], diag[2][:min(i, j)],
                knight[0] if i < 7 and j < 6 else [], knight[1] if i > 0 and j < 6 else [],
                knight[2] if i > 1 and j < 7 else [], knight[3] if i > 1 and j > 0 else [],
                knight[4] if i > 0 and j > 1 else [], knight[5] if i < 7 and j > 1 else [],
                knight[6] if i < 6 and j > 0 else [], knight[7] if i < 6 and j < 7 else [],
                pawn_promotion[0] if i == 6 and j > 0 else [],
                pawn_promotion[1] if i == 6 else [],
                pawn_promotion[2] if i == 6 and j < 7 else [])))))
    z = np.zeros((64 * 64 + 8 * 24, 1858), dtype=np.int32)
    i = 0
    for pickup_index, putdown_indices in enumerate(traversable):
        for putdown_index in putdown_indices:
            if putdown_index < 64:
                z[putdown_index + 64 * pickup_index, i] = 1
                i += 1
    j = 0
    j1 = np.array([3, -2, 3, -2, 3])
    j2 = np.array([3, 3, -5, 3, 3, -5, 3, 3, 1])
    ls = np.append(j1, 1)
    for k in range(6):
        ls = np.append(ls, j2)
    ls = np.append(ls, j1)
    ls = np.append(ls, 0)
    for pickup_index, putdown_indices in enumerate(traversable):
        for putdown_index in putdown_indices:
            if putdown_index >= 64:
                pickup_file = pickup_index % 8
                promotion_file = putdown_index % 8
                promotion_rank = putdown_index // 8 - 8
                z[4096 + pickup_file * 24 + (promotion_file * 3 + promotion_rank), i] = 1
                i += ls[j]
                j += 1
    assert (z.sum(axis=0) == 1).all()
    return z.argmax(axis=0).astype(np.int64)


def _plan(src):
    """Per feature-tile column ranges + matmul segments split at PSUM banks."""
    tiles = []     # (t, cmin, cmax)
    segs = []      # (t, clo, chi)  chi-clo <= 512, within one psum bank group
    for t in range(KT):
        cols = np.nonzero((src >= P * t) & (src < P * (t + 1)))[0]
        if len(cols) == 0:
            continue
        cmin, cmax = int(cols.min()), int(cols.max()) + 1
        assert len(cols) == cmax - cmin, "columns per feature tile not contiguous"
        tiles.append((t, cmin, cmax))
        lo = cmin
        while lo < cmax:
            hi = min(cmax, (lo // 512 + 1) * 512)
            segs.append((t, lo, hi))
            lo = hi
    return tiles, segs


def _build_graph(src, repeat=1):
    """Build + compile the 8-core SPMD Bacc graph."""
    tiles, segs = _plan(src)
    nc = bacc.Bacc("TRN2", target_bir_lowering=False, debug=False,
                   num_devices=N_CORES)
    f32 = mybir.dt.float32
    dt = COMPUTE_DT
    logits_d = nc.dram_tensor("logits", [B_CORE, 4096], f32, kind="ExternalInput").ap()
    pp_d = nc.dram_tensor("pp", [B_CORE, 192], f32, kind="ExternalInput").ap()
    sel_d = nc.dram_tensor("sel", [P, NPOL], dt, kind="ExternalInput").ap()
    ident_d = nc.dram_tensor("ident", [P, P], dt, kind="ExternalInput").ap()
    out_d = nc.dram_tensor("out", [B_CORE, NPOL], OUT_DT, kind="ExternalOutput").ap()

    cast_dma = nc.gpsimd if dt != f32 else nc.sync

    with tile.TileContext(nc) as tc:
        with (
            tc.tile_pool(name="const", bufs=1) as const_pool,
            tc.tile_pool(name="flat", bufs=3) as flat_pool,
            tc.tile_pool(name="flatT", bufs=3) as flatT_pool,
            tc.tile_pool(name="outs", bufs=3) as outs_pool,
            tc.tile_pool(name="psumT", bufs=2, space="PSUM") as psumT_pool,
            tc.tile_pool(name="psumO", bufs=6, space="PSUM") as psumO_pool,
        ):
            sel_t = const_pool.tile([P, NPOL], dt, tag="sel")
            nc.sync.dma_start(sel_t[:], sel_d[:])
            ident_t = const_pool.tile([P, P], dt, tag="ident")
            nc.sync.dma_start(ident_t[:], ident_d[:])

            ngroups = (len(tiles) + GROUP - 1) // GROUP
            for bt_rep in range(NBT * repeat):
                bt = bt_rep % NBT
                r0 = bt * P
                flat = flat_pool.tile([P, F], dt)
                if bt_rep == 0:
                    # chunked first load so PE can start on group 0 early
                    for c0 in range(0, 4096, 1024):
                        cast_dma.dma_start(flat[:, c0:c0 + 1024],
                                           logits_d[r0:r0 + P, c0:c0 + 1024])
                else:
                    cast_dma.dma_start(flat[:, 0:4096], logits_d[r0:r0 + P, :])
                cast_dma.dma_start(flat[:, 4096:F], pp_d[r0:r0 + P, :])

                out_sb = outs_pool.tile([P, NPOL], OUT_DT)
                psumO = [psumO_pool.tile([P, 512], f32, tag="po",
                                         name=f"psumO{q}_{bt_rep}")
                         for q in range(4)]

                for g in range(ngroups):
                    chunk = tiles[g * GROUP:(g + 1) * GROUP]
                    psumT = psumT_pool.tile([P, GROUP * P], dt)
                    for j, (t, _, _) in enumerate(chunk):
                        wt = min(P, F - t * P)
                        nc.tensor.transpose(psumT[:wt, j * P:(j + 1) * P],
                                            flat[:, t * P:t * P + wt],
                                            ident_t[:])
                    flatT = flatT_pool.tile([P, GROUP * P], dt)
                    full = [j for j, (t, _, _) in enumerate(chunk)
                            if min(P, F - t * P) == P]
                    if full:
                        w = (max(full) + 1) * P
                        nc.vector.tensor_copy(flatT[:, :w], psumT[:, :w])
                    for j, (t, _, _) in enumerate(chunk):
                        wt = min(P, F - t * P)
                        if wt < P:
                            nc.vector.tensor_copy(
                                flatT[:wt, j * P:(j + 1) * P],
                                psumT[:wt, j * P:(j + 1) * P])
                    for j, (t, _, _) in enumerate(chunk):
                        wt = min(P, F - t * P)
                        for (tt, clo, chi) in segs:
                            if tt != t:
                                continue
                            q = clo // 512
                            nc.tensor.matmul(
                                psumO[q][:, clo - q * 512:chi - q * 512],
                                lhsT=flatT[:wt, j * P:(j + 1) * P],
                                rhs=sel_t[:wt, clo:chi],
                                start=True, stop=True)

                for q in range(4):
                    wq = min(512, NPOL - q * 512)
                    nc.scalar.copy(out_sb[:, q * 512:q * 512 + wq],
                                   psumO[q][:, :wq])
                nc.sync.dma_start(out_d[r0:r0 + P, :], out_sb[:])

    nc.compile()
    return nc


_CACHE = {}


def _get_graph(src, repeat=1):
    key = (src.tobytes(), repeat)
    if key not in _CACHE:
        _CACHE[key] = _build_graph(src, repeat)
    return _CACHE[key]


def kernel(logits, pp_logits, fc1=None, **_ignored):
    assert logits.shape == (B, 64, 64) and pp_logits.shape == (B, 8, 24)
    if fc1 is not None:
        f = np.asarray(fc1)
        cs = (f != 0).sum(axis=0)
        assert (cs == 1).all(), "fc1 is not a one-hot column map"
        src = np.argmax(f != 0, axis=0).astype(np.int64)
    else:
        src = _make_src()
    nc = _get_graph(src)

    np_dt = _NP_DT[COMPUTE_DT]
    sel = np.zeros((P, NPOL), dtype=np_dt)
    sel[src % P, np.arange(NPOL)] = 1
    ident = np.eye(P, dtype=np_dt)

    lg = np.ascontiguousarray(np.asarray(logits, dtype=np.float32).reshape(B, 4096))
    pp = np.ascontiguousarray(np.asarray(pp_logits, dtype=np.float32).reshape(B, 192))
    in_maps = [{
        "logits": lg[i * B_CORE:(i + 1) * B_CORE],
        "pp": pp[i * B_CORE:(i + 1) * B_CORE],
        "sel": sel,
        "ident": ident,
    } for i in range(N_CORES)]

    res = run_bass_kernel_spmd(nc, in_maps, core_ids=list(range(N_CORES)))
    out = np.concatenate([res.results[i]["out"] for i in range(N_CORES)], axis=0)
    return out.astype(np.float32)
